# revision 17
# baseline (speedup 1.0000x reference)
"""ALiBi attention (B=2, S=2048, H=16, Dh=64) on 8 TRN2 NeuronCores.

Sharding: head-parallel attention (2 heads x 2 batches per core), qkv
column-sharded, out-projection K-sharded (per-core partial summed on host).
No collectives. All heavy matmuls in bf16 with f32 PSUM accumulation.

One SPMD graph shared by all 8 cores: everything slope/head-dependent
(exp(ALiBi-bias) tiles) arrives via per-core inputs.

Softmax is computed in transposed score layout [k,q]; the denominator
falls out of the PV matmul via a mask column appended to V. The ALiBi
bias is applied multiplicatively AFTER exp (probs = exp(s)*exp(bias))
on the otherwise-idle GPSIMD engine; exp(bias) tiles are host-precomputed
per (head, tile-diagonal-offset) — multiplicative bf16 error is ~0.4%
with no cancellation hazard.
"""

import math
import numpy as np
import ml_dtypes

bf16 = ml_dtypes.bfloat16

HID, H, DH = 1024, 16, 64
B, S = 2, 2048
NCORES = 8
NEB = 16  # exp-bias tile-pair classes: 4 mixed (delta 0..-384) + 12 linear (128..1536)


def _alibi_slopes(n_head):
    main = 2 ** int(math.log2(n_head))
    m = (2.0 ** (-8.0 / main)) ** np.arange(1, 1 + main)
    if main < n_head:
        intra = (2.0 ** (-4.0 / main)) ** np.arange(1, 1 + 2 * (n_head - main), 2)
        m = np.concatenate([m, intra])
    return m.astype(np.float32)


def _eb_idx(delta):
    """exp-bias class index for tile diagonal offset delta, or None if bias==0."""
    if delta <= -512:
        return None
    if delta >= 128:
        return delta // 128 + 3  # 4..15
    return (-delta) // 128  # 0..3


def build_nc():
    import concourse.tile as tile
    from concourse import bacc, mybir

    f32 = mybir.dt.float32
    bf = mybir.dt.bfloat16
    AF = mybir.ActivationFunctionType

    nc = bacc.Bacc("TRN2", target_bir_lowering=False, debug=False,
                   enable_asserts=False, num_devices=NCORES)

    # ---- DRAM I/O ----
    xT_d = nc.dram_tensor("xT", [8, 128, 4096], bf, kind="ExternalInput").ap()
    wq_d = nc.dram_tensor("wq", [8, 128, 128], bf, kind="ExternalInput").ap()
    wk_d = nc.dram_tensor("wk", [8, 128, 128], bf, kind="ExternalInput").ap()
    wv_d = nc.dram_tensor("wv", [8, 128, 128], bf, kind="ExternalInput").ap()
    wout_d = nc.dram_tensor("wout", [128, 1024], bf, kind="ExternalInput").ap()
    eb_d = nc.dram_tensor("ebias", [128, NEB * 1024], bf, kind="ExternalInput").ap()
    mk_d = nc.dram_tensor("maskf", [128, 32], f32, kind="ExternalInput").ap()

    out_d = nc.dram_tensor("out_p", [2, 16, 128, 1024], bf, kind="ExternalOutput").ap()
    k_d = nc.dram_tensor("k_out", [128, 4096], bf, kind="ExternalOutput").ap()
    v_d = nc.dram_tensor("v_out", [128, 32, 128], bf, kind="ExternalOutput").ap()

    with tile.TileContext(nc) as tc:
        import contextlib
        with contextlib.ExitStack() as ctx:
            persist = ctx.enter_context(tc.tile_pool(name="persist", bufs=1))
            qT = persist.tile([128, 4096], bf, tag="qT")
            kT = persist.tile([128, 4096], bf, tag="kT")
            v_sb = persist.tile([128, 32 * 130], bf, tag="v_sb")
            attn_T = persist.tile([128, 4096], bf, tag="attn_T")
            wout_sb = persist.tile([128, 1024], bf, tag="wout")
            eb_sb = persist.tile([128, NEB * 1024], bf, tag="eb")
            maskf = persist.tile([128, 32], f32, tag="maskf")
            atst = persist.tile([64, 8 * 512], f32, tag="atst")   # numerators (1 batch)
            # denominators: slot (qc,hl) -> partition 32*qc, col hl*512
            # (compute writes must start at 32-aligned partitions)
            den = persist.tile([128, 1024], f32, tag="den")
            rcp = persist.tile([128, 1024], f32, tag="rcp")
            ones_sb = persist.tile([1, 64], f32, tag="ones")

            xw = ctx.enter_context(tc.tile_pool(name="xw", bufs=1))
            xT_sb = xw.tile([128, 8 * 4096], bf, tag="xT")
            wq_sb = xw.tile([128, 8 * 128], bf, tag="wq")
            wk_sb = xw.tile([128, 8 * 128], bf, tag="wk")
            wv_sb = xw.tile([128, 8 * 128], bf, tag="wv")

            work = ctx.enter_context(tc.tile_pool(name="work", bufs=4))
            work2 = ctx.enter_context(tc.tile_pool(name="work2", bufs=2))
            ps_sc = ctx.enter_context(tc.tile_pool(name="ps_sc", bufs=2, space="PSUM"))
            ps_ot = ctx.enter_context(tc.tile_pool(name="ps_ot", bufs=2, space="PSUM"))
            ps_ms = ctx.enter_context(tc.tile_pool(name="ps_ms", bufs=2, space="PSUM"))

            # ---- input DMAs (weights first, then x for batch 0, then rest) ----
            for h in range(8):
                nc.sync.dma_start(wq_sb[:, h * 128:(h + 1) * 128], wq_d[h])
                nc.sync.dma_start(wk_sb[:, h * 128:(h + 1) * 128], wk_d[h])
                nc.sync.dma_start(wv_sb[:, h * 128:(h + 1) * 128], wv_d[h])
            nc.sync.dma_start(maskf[:], mk_d[:])
            for n in range(4):  # batch-0 row slices, finest first
                for h in range(8):
                    nc.sync.dma_start(
                        xT_sb[:, h * 4096 + n * 512: h * 4096 + (n + 1) * 512],
                        xT_d[h, :, n * 512:(n + 1) * 512])
            nc.sync.dma_start(wout_sb[:], wout_d[:])
            for h in range(8):
                nc.sync.dma_start(xT_sb[:, h * 4096 + 2048: (h + 1) * 4096],
                                  xT_d[h, :, 2048:4096])
            # eb on the gpsimd queue: parallel to the sync-queue input stream
            for j in range(8):
                nc.gpsimd.dma_start(eb_sb[:, j * 2048:(j + 1) * 2048],
                                    eb_d[:, j * 2048:(j + 1) * 2048])

            # mask columns of v_sb (positions 64 and 129 of each 130-chunk)
            v3 = v_sb.rearrange("p (c w) -> p c w", w=130)
            mk3 = maskf.rearrange("p (c o) -> p c o", o=1)
            nc.vector.tensor_copy(v3[:, :, 64:65], mk3[:])
            nc.vector.tensor_copy(v3[:, :, 129:130], mk3[:])
            nc.vector.memset(den[:], 1.0)  # unused slots stay finite for reciprocal
            nc.vector.memset(ones_sb[:], 1.0)

            def qkv_phase(b):
                for w_sb, dst in ((wq_sb, qT), (wk_sb, kT)):
                    for n in range(4 * b, 4 * b + 4):  # 512-row chunks
                        ps = ps_ms.tile([128, 512], f32, tag="ms", name=f"qk{b}_{n}")
                        for h in range(8):
                            nc.tensor.matmul(
                                ps[:],
                                lhsT=w_sb[:, h * 128:(h + 1) * 128],
                                rhs=xT_sb[:, h * 4096 + n * 512: h * 4096 + (n + 1) * 512],
                                start=(h == 0), stop=(h == 7))
                        nc.vector.tensor_copy(
                            dst[:, n * 512:(n + 1) * 512], ps[:])
                vo = None
                for r in range(16 * b, 16 * b + 16):  # 128-row chunks
                    if r % 4 == 0:
                        vo = work2.tile([128, 512], bf, tag="vo", name=f"vo{r}")
                    ps = ps_ms.tile([128, 128], f32, tag="ms", name=f"v{b}_{r}")
                    for h in range(8):
                        nc.tensor.matmul(
                            ps[:],
                            lhsT=xT_sb[:, h * 4096 + r * 128: h * 4096 + r * 128 + 128],
                            rhs=wv_sb[:, h * 128:(h + 1) * 128],
                            start=(h == 0), stop=(h == 7))
                    for hl in range(2):
                        nc.vector.tensor_scalar_mul(
                            v3[:, r, hl * 65: hl * 65 + 64],
                            ps[:, hl * 64:(hl + 1) * 64],
                            maskf[:, r:r + 1])
                    nc.any.tensor_copy(vo[:, (r % 4) * 128:(r % 4) * 128 + 128], ps[:])
                    if r % 4 == 3:
                        nc.sync.dma_start(v_d[:, r - 3:r + 1, :],
                                          vo.rearrange("p (c w) -> p c w", w=128)[:])

            def attention_phase(b, qcs=range(4)):
                for qc in qcs:
                    q0 = b * 2048 + qc * 512
                    otiles = [ps_ot.tile([65, 512], f32, tag="ot", name=f"ot{b}_{qc}_{hl}")
                              for hl in range(2)]
                    for kc in range(16):
                        delta = qc * 512 - kc * 128
                        ei = _eb_idx(delta)
                        sw = ps_sc.tile([128, 1024], f32, tag="sc", name=f"sw{b}_{qc}_{kc}")
                        koff = b * 2048 + kc * 128
                        qoff = b * 2048 + qc * 512
                        for hl in range(2):
                            nc.tensor.matmul(
                                sw[:, hl * 512:(hl + 1) * 512],
                                lhsT=kT[hl * 64:(hl + 1) * 64, koff:koff + 128],
                                rhs=qT[hl * 64:(hl + 1) * 64, qoff:qoff + 512],
                                start=True, stop=True)
                        pw = work.tile([128, 1024], bf, tag="pw", name=f"pw{b}_{qc}_{kc}")
                        nc.scalar.activation(pw[:], sw[:], AF.Exp)
                        if ei is not None:
                            pb = work.tile([128, 1024], bf, tag="pb", name=f"pb{b}_{qc}_{kc}")
                            eng = nc.gpsimd if kc % 3 == 0 else nc.vector
                            eng.tensor_mul(pb[:], pw[:],
                                           eb_sb[:, ei * 1024:(ei + 1) * 1024])
                            src = pb
                        else:
                            src = pw
                        ch = b * 16 + kc
                        for hl in range(2):
                            nc.tensor.matmul(
                                otiles[hl][:],
                                lhsT=v_sb[:, ch * 130 + hl * 65: ch * 130 + hl * 65 + 65],
                                rhs=src[:, hl * 512:(hl + 1) * 512],
                                start=(kc == 0), stop=(kc == 15))
                    for hl in range(2):
                        slot = qc * 2 + hl
                        nc.vector.tensor_copy(
                            den[32 * qc:32 * qc + 1, hl * 512:(hl + 1) * 512],
                            otiles[hl][64:65, :])
                        nc.vector.tensor_copy(atst[:, slot * 512:(slot + 1) * 512],
                                              otiles[hl][0:64, :])

            def normalize_and_outproj(b, half=None):
                qcs = range(4) if half is None else range(2 * half, 2 * half + 2)
                p0 = 0 if (half is None or half == 0) else 64
                np_ = 128 if half is None else 64
                nc.vector.reciprocal(rcp[p0:p0 + np_, :], den[p0:p0 + np_, :])
                for qc in qcs:
                    q0 = b * 2048 + qc * 512
                    for hl in range(2):
                        slot = qc * 2 + hl
                        rc0 = work.tile([1, 512], f32, tag="rc0", name=f"rc{b}_{slot}")
                        nc.vector.tensor_copy(
                            rc0[:], rcp[32 * qc:32 * qc + 1, hl * 512:(hl + 1) * 512])
                        bc = ps_ms.tile([64, 512], f32, tag="ms", name=f"bc{b}_{slot}")
                        nc.tensor.matmul(bc[:], lhsT=ones_sb[:], rhs=rc0[:],
                                         start=True, stop=True)
                        nc.vector.tensor_mul(
                            attn_T[hl * 64:(hl + 1) * 64, q0:q0 + 512],
                            atst[:, slot * 512:(slot + 1) * 512], bc[:])
                rr = range(16) if half is None else range(8 * half, 8 * half + 8)
                for r in rr:
                    st = work.tile([128, 1024], bf, tag="ostage", name=f"st{b}_{r}")
                    for n2 in range(2):
                        ps = ps_ms.tile([128, 512], f32, tag="ms", name=f"op{b}_{r}_{n2}")
                        nc.tensor.matmul(
                            ps[:],
                            lhsT=attn_T[:, b * 2048 + r * 128: b * 2048 + r * 128 + 128],
                            rhs=wout_sb[:, n2 * 512:(n2 + 1) * 512],
                            start=True, stop=True)
                        nc.any.tensor_copy(st[:, n2 * 512:(n2 + 1) * 512], ps[:])
                    nc.sync.dma_start(out_d[b, r], st[:])

            qkv_phase(0)
            attention_phase(0)
            qkv_phase(1)
            nc.sync.dma_start(k_d[:], kT[:])
            normalize_and_outproj(0)
            attention_phase(1, range(0, 2))
            normalize_and_outproj(1, half=0)
            attention_phase(1, range(2, 4))
            normalize_and_outproj(1, half=1)

    nc.compile()
    return nc


_COMPILED = None


def _get_nc():
    global _COMPILED
    if _COMPILED is None:
        _COMPILED = build_nc()
    return _COMPILED


def _host_inputs(x, mask, Wqkv, Wout):
    slopes = _alibi_slopes(H)
    xf = np.ascontiguousarray(np.asarray(x, np.float32).reshape(B * S, HID))
    xT = np.ascontiguousarray(xf.T).astype(bf16).reshape(8, 128, 4096)
    maskf = np.asarray(mask, np.float32).reshape(B * S)
    mk = np.ascontiguousarray(maskf.reshape(32, 128).T)  # [128p, 32c]

    kk = np.arange(128, dtype=np.float32)[:, None]
    qq = np.arange(512, dtype=np.float32)[None, :]
    base = qq - kk  # [128, 512]

    Wqkv = np.asarray(Wqkv, np.float32)
    Wout = np.asarray(Wout, np.float32)

    in_maps = []
    for c in range(NCORES):
        h0 = 2 * c
        wq = np.ascontiguousarray(Wqkv[:, h0 * 64: h0 * 64 + 128]
                                  ).astype(bf16).reshape(8, 128, 128)
        wk = np.ascontiguousarray(Wqkv[:, 1024 + h0 * 64: 1024 + h0 * 64 + 128]
                                  ).astype(bf16).reshape(8, 128, 128)
        wv = np.ascontiguousarray(Wqkv[:, 2048 + h0 * 64: 2048 + h0 * 64 + 128]
                                  ).astype(bf16).reshape(8, 128, 128)
        wo = np.ascontiguousarray(Wout[c * 128:(c + 1) * 128, :]).astype(bf16)
        eb = np.empty((128, NEB * 1024), np.float32)
        for hl in range(2):
            sl = float(slopes[h0 + hl])
            for idx in range(NEB):
                if idx < 4:
                    bias = -sl * np.maximum(base - 128.0 * idx, 0.0)
                else:
                    bias = -sl * (base + 128.0 * (idx - 3))
                eb[:, idx * 1024 + hl * 512: idx * 1024 + (hl + 1) * 512] = np.exp(bias)
        in_maps.append({
            "xT": xT, "wq": wq, "wk": wk, "wv": wv, "wout": wo,
            "ebias": eb.astype(bf16), "maskf": mk,
        })
    return in_maps


def kernel(x, mask, Wqkv, Wout, trace=False):
    from concourse.bass_utils import run_bass_kernel_spmd

    nc = _get_nc()
    in_maps = _host_inputs(x, mask, Wqkv, Wout)
    res = run_bass_kernel_spmd(nc, in_maps, core_ids=list(range(NCORES)), trace=trace)
    results = res.results

    out = np.zeros((B * S, HID), np.float32)
    k_cache = np.empty((B * S, HID), np.float32)
    v_cache = np.empty((B * S, HID), np.float32)
    for c in range(NCORES):
        r = results[c]
        out += r["out_p"].astype(np.float32).reshape(B * S, HID)
        k_cache[:, c * 128:(c + 1) * 128] = r["k_out"].astype(np.float32).T
        v_cache[:, c * 128:(c + 1) * 128] = \
            r["v_out"].astype(np.float32).transpose(1, 0, 2).reshape(B * S, 128)
    out = out.reshape(B, S, HID)
    k_cache = k_cache.reshape(B, S, HID)
    v_cache = v_cache.reshape(B, S, HID)
    if trace:
        return (out, k_cache, v_cache), res
    return out, k_cache, v_cache
